# revision 11
# baseline (speedup 1.0000x reference)
"""Trainium2 Bass kernel for AetherLoss: chamfer(recon_x, x) + beta*KL(mu, logvar).

Strategy (v5 "softmin hybrid", data-parallel over batch B=8 across 8 cores):

Host prep: the fp32->3x-bf16 augmented operands AX/AY [24, 4096] (6 split-pair
blocks for 2x.y + norm trios vs ones) are built in numpy, so the device does no
operand prep.  Per core the PE produces the negated squared-distance matrix
-d[x, y] in fp32 PSUM, 4 N=512 matmuls per [128, 2048] half-tile, double
buffered across the 8 PSUM banks.

Reductions (the engine-balance core of the design):
- 27 "a-tiles": ScalarE evacuates each half-tile as exp(S * -d) -> bf16 SBUF
  with a FUSED free-axis accumulator (accum_out), which IS the row softmin
  (sum of exps per row, log-ed on the host).  DVE then folds the staged exp
  tile into a running column-max accumulator at 16-bit 2x rate.  Max in exp
  domain == min distance (monotone), so the column path needs no extra math.
- 5 "v-tiles" ({5,11,17,23,29}): DVE consumes the PSUM directly (1x fp32):
  tensor_tensor max into a raw fp16 column accumulator + tensor_scalar with
  accum_out(max) for exact row maxes.  This offloads ScalarE just enough to
  balance ScalarE ~= DVE ~= 108us.

Host combine: rows = -ln(rowsum)/S (a-tiles, softmin S=1600) or -rowmax
(v-tiles); cols = min over the exp-domain and raw column accumulators
(partition-axis max done in numpy on the DMA-ed [128, 4096] accumulators).
Rows/cols whose exp-domain signal underflowed (sum < e^-60, i.e. min dist
> ~0.0375) are recomputed exactly on the host (~5-10% of rows/cols, a few
ms of numpy) - softmin bias and bf16 underflow only affect far-outlier
points, and this rescue makes them exact.  Validated end-to-end in numpy:
rel err ~1.1e-3 on the worst-case data flavor.
"""

import numpy as np
import ml_dtypes
from contextlib import ExitStack

B, D, N = 8, 3, 4096
LATENT = 256
NCORES = 8
BETA = 1.0

K = 24              # augmented contraction size
PT = 128            # x-tile size
NT = N // PT        # 32 x-tiles
HC = 2048           # psum half-tile free size (4 banks)
CH = 512            # matmul moving free dim (1 psum bank)

S = 1600.0          # softmin sharpness
LN_THRESH = -60.0   # host-rescue threshold on ln(signal)
TAU0 = -LN_THRESH / S           # softmin reliability bound on dist
SEG = 512           # DVE-direct segment (last PSUM bank of a mixed half)


def _is_mixed(hg):
    """9 of every 16 halves give their last bank to the DVE-direct route."""
    return (hg * 9) % 16 < 9

bf16 = ml_dtypes.bfloat16

_cache = {}


def _split3(v):
    h = v.astype(bf16)
    m = (v - h.astype(np.float64)).astype(bf16)
    l = (v - h.astype(np.float64) - m.astype(np.float64)).astype(bf16)
    return h, m, l


def build_aug(x, y):
    """x, y: [3, N] float64 -> AX, AY [24, N] bf16."""
    axh, axm, axl = _split3(2.0 * x)
    yh, ym, yl = _split3(y)
    x2h, x2m, x2l = _split3(-(x * x).sum(0)[None, :])
    y2h, y2m, y2l = _split3(-(y * y).sum(0)[None, :])
    ones = np.ones((3, x.shape[1]), dtype=bf16)
    AX = np.concatenate([
        axh, axh, axm, axh, axl, axm,
        np.concatenate([x2h, x2m, x2l], 0), ones], 0).astype(bf16)
    AY = np.concatenate([
        yh, ym, yh, yl, yh, ym,
        ones, np.concatenate([y2h, y2m, y2l], 0)], 0).astype(bf16)
    return AX, AY


def _build_program():
    import concourse.bass as bass
    import concourse.tile as tile
    from concourse import bacc, mybir

    f32 = mybir.dt.float32
    f16 = mybir.dt.float16
    bf = mybir.dt.bfloat16
    MAX = mybir.AluOpType.max
    MULT = mybir.AluOpType.mult

    nc = bacc.Bacc(trn_type="TRN2", debug=False, target_bir_lowering=False)

    ax = nc.dram_tensor("ax", [K, N], bf, kind="ExternalInput")
    ay = nc.dram_tensor("ay", [K, N], bf, kind="ExternalInput")
    mu = nc.dram_tensor("mu", [LATENT], f32, kind="ExternalInput")
    lv = nc.dram_tensor("lv", [LATENT], f32, kind="ExternalInput")

    o_cexp = nc.dram_tensor("o_cexp", [128, N], bf, kind="ExternalOutput")
    o_craw = nc.dram_tensor("o_craw", [128, N], f16, kind="ExternalOutput")
    o_rs = nc.dram_tensor("o_rs", [128, 2 * NT], f32, kind="ExternalOutput")
    o_rmv = nc.dram_tensor("o_rmv", [128, 2 * NT], f32, kind="ExternalOutput")
    o_kl = nc.dram_tensor("o_kl", [128, 1], f32, kind="ExternalOutput")

    with tile.TileContext(nc) as tc, ExitStack() as ctx:
        const = ctx.enter_context(tc.tile_pool(name="const", bufs=1))
        work = ctx.enter_context(tc.tile_pool(name="work", bufs=1))
        stg = ctx.enter_context(tc.tile_pool(name="stg", bufs=3))
        psum = ctx.enter_context(tc.tile_pool(name="psum", bufs=2, space="PSUM"))

        axs = const.tile([K, N], bf, tag="axs")
        ays = const.tile([K, N], bf, tag="ays")
        nc.sync.dma_start(axs[:], ax.ap())
        nc.gpsimd.dma_start(ays[:], ay.ap())

        # ---- accumulators (memset on otherwise-idle GpSimd) ----
        colacc_exp = const.tile([128, N], bf, tag="colacc_exp")
        colacc_raw = const.tile([128, N], f16, tag="colacc_raw")
        rs_t = const.tile([128, 2 * NT], f32, tag="rs_t")
        rmv_t = const.tile([128, 2 * NT], f32, tag="rmv_t")
        junk = work.tile([128, SEG], f16, tag="junk")
        nc.gpsimd.memset(colacc_exp[:], 0.0)
        nc.gpsimd.memset(colacc_raw[:], -60000.0)
        nc.gpsimd.memset(rs_t[:], 0.0)
        nc.gpsimd.memset(rmv_t[:], -60000.0)

        # ---- main loop ----
        AW = HC - SEG   # ScalarE (softmin) width of a mixed half
        for pt in range(NT):
            for h in range(2):
                hg = 2 * pt + h
                mixed = _is_mixed(hg)
                ptile = psum.tile([128, HC], f32, tag="ptile",
                                  name=f"pt{pt}_{h}")
                for q in range(4):
                    nc.tensor.matmul(
                        ptile[:, q * CH:(q + 1) * CH],
                        axs[0:K, pt * PT:(pt + 1) * PT],
                        ays[0:K, h * HC + q * CH:h * HC + (q + 1) * CH],
                        start=True, stop=True,
                    )
                w = AW if mixed else HC
                exph = stg.tile([128, HC], bf, tag="exph", name=f"exph{hg}")
                nc.scalar.activation(
                    exph[:, 0:w], ptile[:, 0:w],
                    mybir.ActivationFunctionType.Exp, scale=S,
                    accum_out=rs_t[:, hg:hg + 1])
                nc.vector.tensor_tensor(
                    colacc_exp[:, h * HC:h * HC + w],
                    colacc_exp[:, h * HC:h * HC + w],
                    exph[:, 0:w], op=MAX)
                if mixed:
                    # DVE consumes the last bank directly (parallel banks)
                    nc.vector.tensor_tensor(
                        colacc_raw[:, h * HC + AW:(h + 1) * HC],
                        colacc_raw[:, h * HC + AW:(h + 1) * HC],
                        ptile[:, AW:HC], op=MAX)
                    nc.vector.tensor_scalar(
                        junk[:], ptile[:, AW:HC], 1.0, None, MULT,
                        op1=MAX, accum_out=rmv_t[:, hg:hg + 1])

        # ---- KL term (tiny) ----
        mu2d = work.tile([128, LATENT // 128], f32, tag="mu2d")
        lv2d = work.tile([128, LATENT // 128], f32, tag="lv2d")
        nc.sync.dma_start(mu2d[:], mu.ap().rearrange("(p f) -> p f", p=128))
        nc.sync.dma_start(lv2d[:], lv.ap().rearrange("(p f) -> p f", p=128))
        klsq = work.tile([128, LATENT // 128], f32, tag="klsq")
        klex = work.tile([128, LATENT // 128], f32, tag="klex")
        klt = work.tile([128, LATENT // 128], f32, tag="klt")
        klp = work.tile([128, 1], f32, tag="klp")
        nc.vector.tensor_tensor(klsq[:], mu2d[:], mu2d[:], op=MULT)
        nc.scalar.activation(klex[:], lv2d[:], mybir.ActivationFunctionType.Exp)
        nc.vector.tensor_tensor(klt[:], lv2d[:], klsq[:],
                                op=mybir.AluOpType.subtract)
        nc.vector.tensor_tensor(klt[:], klt[:], klex[:],
                                op=mybir.AluOpType.subtract)
        nc.vector.reduce_sum(klp[:], klt[:], axis=mybir.AxisListType.X)
        nc.sync.dma_start(o_kl.ap(), klp[:])

        # ---- outputs ----
        nc.sync.dma_start(o_cexp.ap(), colacc_exp[:])
        nc.gpsimd.dma_start(o_craw.ap(), colacc_raw[:])
        nc.sync.dma_start(o_rs.ap(), rs_t[:])
        nc.gpsimd.dma_start(o_rmv.ap(), rmv_t[:])

    nc.compile()
    return nc


def _get_nc():
    if "nc" not in _cache:
        _cache["nc"] = _build_program()
    return _cache["nc"]


def _register_ntff_hook():
    import sys, types
    if "antenv.axon_hooks" in sys.modules:
        return
    try:
        from trn_agent_boot.trn_boot import _ntff_profile_via_ctypes
        hook = _ntff_profile_via_ctypes("/opt/axon/libaxon_pjrt.so")
        mod = types.ModuleType("antenv.axon_hooks")
        mod.get_axon_ntff_profile_hook = lambda: hook
        mod.set_axon_ntff_profile_hook = lambda h: None
        sys.modules["antenv.axon_hooks"] = mod
        from concourse import bass_utils
        bass_utils.upload_artifacts = lambda tmpdir: tmpdir
    except Exception:
        pass


def _run(in_maps, trace=False):
    from concourse.bass_utils import run_bass_kernel_spmd
    if trace:
        _register_ntff_hook()
    nc = _get_nc()
    return run_bass_kernel_spmd(nc, in_maps, list(range(NCORES)), trace=trace)


def _combine(results, recon_x, x):
    """Host-side finish: logs, rescue of underflowed rows/cols, means, KL."""
    thresh = np.exp(LN_THRESH)
    row_total = 0.0
    col_total = 0.0
    kl_sum = 0.0
    for c in range(NCORES):
        r = results[c]
        xs = recon_x[c].astype(np.float64)   # [3, N] row points
        ys = x[c].astype(np.float64)         # [3, N] col points

        # ---- rows: softmin part per half + exact raw segment (mixed) ----
        rs = r["o_rs"].astype(np.float64)    # [128, 64]
        rmv = r["o_rmv"].astype(np.float64)  # [128, 64]
        with np.errstate(divide="ignore"):
            d_soft = -np.log(np.maximum(rs, 1e-300)) / S     # [128, 64]
        d_raw = -rmv                         # 60000.0 where not mixed
        unreliable = rs < thresh
        d_soft_ok = np.where(unreliable, np.inf, d_soft)
        d_half_ok = np.minimum(d_soft_ok, d_raw)
        # [128, 32, 2] -> per-row min over the two halves
        dv = d_half_ok.reshape(128, NT, 2).min(2)            # [128, NT]
        any_unrel = unreliable.reshape(128, NT, 2).any(2)
        need = any_unrel & (dv > TAU0)
        # row index = pt*128 + p
        rowvals = np.ascontiguousarray(dv.transpose(1, 0)).reshape(N)
        if need.any():
            p_idx, t_idx = np.nonzero(need)
            idx = t_idx * PT + p_idx
            xr = xs[:, idx]                  # [3, R]
            d = ((xr * xr).sum(0)[:, None] + (ys * ys).sum(0)[None, :]
                 - 2.0 * xr.T @ ys)          # [R, N]
            rowvals[idx] = d.min(1)

        # ---- cols ----
        cexp = r["o_cexp"].astype(np.float64).max(0)   # [N]
        craw = r["o_craw"].astype(np.float64).max(0)   # [N]
        d_raw = -craw
        with np.errstate(divide="ignore"):
            d_exp = np.where(cexp > 0.0,
                             -np.log(np.maximum(cexp, 1e-300)) / S, np.inf)
        colvals = np.minimum(d_exp, d_raw)
        badc = (cexp < thresh) & (colvals > TAU0)
        if badc.any():
            idx = np.nonzero(badc)[0]
            yc = ys[:, idx]
            d = ((xs * xs).sum(0)[:, None] + (yc * yc).sum(0)[None, :]
                 - 2.0 * xs.T @ yc)          # [N, C]
            colvals[idx] = d.min(0)

        row_total += rowvals.mean()
        col_total += colvals.mean()
        kl_sum += r["o_kl"].astype(np.float64).sum()

    recon = (row_total + col_total) / NCORES
    kld = -0.5 * (B * LATENT * 1.0 + kl_sum) / B
    total = recon + BETA * kld
    return (np.float32(total), np.float32(recon), np.float32(kld))


def _prep_in_maps(recon_x, x, mu, logvar):
    in_maps = []
    for c in range(NCORES):
        AX, AY = build_aug(recon_x[c].astype(np.float64),
                           x[c].astype(np.float64))
        in_maps.append({"ax": AX, "ay": AY, "mu": mu[c], "lv": logvar[c]})
    return in_maps


def kernel(recon_x, x, mu, logvar, _trace=False):
    recon_x = np.ascontiguousarray(recon_x, dtype=np.float32)
    x = np.ascontiguousarray(x, dtype=np.float32)
    mu = np.ascontiguousarray(mu, dtype=np.float32)
    logvar = np.ascontiguousarray(logvar, dtype=np.float32)
    in_maps = _prep_in_maps(recon_x, x, mu, logvar)
    res = _run(in_maps, trace=_trace)
    out = _combine(res.results, recon_x, x)
    if _trace:
        return out, res
    return out


# revision 13
# speedup vs baseline: 1.0412x; 1.0412x over previous
"""Trainium2 Bass kernel for AetherLoss: chamfer(recon_x, x) + beta*KL(mu, logvar).

Strategy (v5 "softmin hybrid", data-parallel over batch B=8 across 8 cores):

Host prep: the fp32->3x-bf16 augmented operands AX/AY [24, 4096] (6 split-pair
blocks for 2x.y + norm trios vs ones) are built in numpy, so the device does no
operand prep.  Per core the PE produces the negated squared-distance matrix
-d[x, y] in fp32 PSUM, 4 N=512 matmuls per [128, 2048] half-tile, double
buffered across the 8 PSUM banks.

Reductions (the engine-balance core of the design):
- 27 "a-tiles": ScalarE evacuates each half-tile as exp(S * -d) -> bf16 SBUF
  with a FUSED free-axis accumulator (accum_out), which IS the row softmin
  (sum of exps per row, log-ed on the host).  DVE then folds the staged exp
  tile into a running column-max accumulator at 16-bit 2x rate.  Max in exp
  domain == min distance (monotone), so the column path needs no extra math.
- 5 "v-tiles" ({5,11,17,23,29}): DVE consumes the PSUM directly (1x fp32):
  tensor_tensor max into a raw fp16 column accumulator + tensor_scalar with
  accum_out(max) for exact row maxes.  This offloads ScalarE just enough to
  balance ScalarE ~= DVE ~= 108us.

Host combine: rows = -ln(rowsum)/S (a-tiles, softmin S=1600) or -rowmax
(v-tiles); cols = min over the exp-domain and raw column accumulators
(partition-axis max done in numpy on the DMA-ed [128, 4096] accumulators).
Rows/cols whose exp-domain signal underflowed (sum < e^-60, i.e. min dist
> ~0.0375) are recomputed exactly on the host (~5-10% of rows/cols, a few
ms of numpy) - softmin bias and bf16 underflow only affect far-outlier
points, and this rescue makes them exact.  Validated end-to-end in numpy:
rel err ~1.1e-3 on the worst-case data flavor.
"""

import numpy as np
import ml_dtypes
from contextlib import ExitStack

B, D, N = 8, 3, 4096
LATENT = 256
NCORES = 8
BETA = 1.0

K = 24              # augmented contraction size
PT = 128            # x-tile size
NT = N // PT        # 32 x-tiles
HC = 2048           # psum half-tile free size (4 banks)
CH = 512            # matmul moving free dim (1 psum bank)

S = 1600.0          # softmin sharpness
LN_THRESH = -60.0   # host-rescue threshold on ln(signal)
TAU0 = -LN_THRESH / S           # softmin reliability bound on dist
SEG = 512           # DVE-direct segment (last PSUM bank of a mixed half)


def _is_mixed(hg):
    """45 of 64 halves give their last bank to the DVE-direct route."""
    return (hg * 45) % 64 < 45

bf16 = ml_dtypes.bfloat16

_cache = {}


def _split3(v):
    h = v.astype(bf16)
    m = (v - h.astype(np.float64)).astype(bf16)
    l = (v - h.astype(np.float64) - m.astype(np.float64)).astype(bf16)
    return h, m, l


def build_aug(x, y):
    """x, y: [3, N] float64 -> AX, AY [24, N] bf16."""
    axh, axm, axl = _split3(2.0 * x)
    yh, ym, yl = _split3(y)
    x2h, x2m, x2l = _split3(-(x * x).sum(0)[None, :])
    y2h, y2m, y2l = _split3(-(y * y).sum(0)[None, :])
    ones = np.ones((3, x.shape[1]), dtype=bf16)
    AX = np.concatenate([
        axh, axh, axm, axh, axl, axm,
        np.concatenate([x2h, x2m, x2l], 0), ones], 0).astype(bf16)
    AY = np.concatenate([
        yh, ym, yh, yl, yh, ym,
        ones, np.concatenate([y2h, y2m, y2l], 0)], 0).astype(bf16)
    return AX, AY


def _build_program():
    import concourse.bass as bass
    import concourse.tile as tile
    from concourse import bacc, mybir

    f32 = mybir.dt.float32
    f16 = mybir.dt.float16
    bf = mybir.dt.bfloat16
    MAX = mybir.AluOpType.max
    MULT = mybir.AluOpType.mult

    nc = bacc.Bacc(trn_type="TRN2", debug=False, target_bir_lowering=False)

    ax = nc.dram_tensor("ax", [K, N], bf, kind="ExternalInput")
    ay = nc.dram_tensor("ay", [K, N], bf, kind="ExternalInput")
    mu = nc.dram_tensor("mu", [LATENT], f32, kind="ExternalInput")
    lv = nc.dram_tensor("lv", [LATENT], f32, kind="ExternalInput")

    o_cexp = nc.dram_tensor("o_cexp", [128, N], bf, kind="ExternalOutput")
    o_craw = nc.dram_tensor("o_craw", [128, N], f16, kind="ExternalOutput")
    o_rs = nc.dram_tensor("o_rs", [128, 2 * NT], f32, kind="ExternalOutput")
    o_rmv = nc.dram_tensor("o_rmv", [128, 2 * NT], f32, kind="ExternalOutput")
    o_kl = nc.dram_tensor("o_kl", [128, 1], f32, kind="ExternalOutput")

    with tile.TileContext(nc) as tc, ExitStack() as ctx:
        const = ctx.enter_context(tc.tile_pool(name="const", bufs=1))
        work = ctx.enter_context(tc.tile_pool(name="work", bufs=1))
        stg = ctx.enter_context(tc.tile_pool(name="stg", bufs=3))
        psum = ctx.enter_context(tc.tile_pool(name="psum", bufs=2, space="PSUM"))

        axs = const.tile([K, N], bf, tag="axs")
        ays = const.tile([K, N], bf, tag="ays")
        nc.sync.dma_start(axs[:], ax.ap())
        nc.gpsimd.dma_start(ays[:], ay.ap())

        # ---- accumulators (memset on otherwise-idle GpSimd) ----
        colacc_exp = const.tile([128, N], bf, tag="colacc_exp")
        colacc_raw = const.tile([128, N], f16, tag="colacc_raw")
        rs_t = const.tile([128, 2 * NT], f32, tag="rs_t")
        rmv_t = const.tile([128, 2 * NT], f32, tag="rmv_t")
        junk = work.tile([128, SEG], f16, tag="junk")
        nc.gpsimd.memset(colacc_exp[:], 0.0)
        nc.gpsimd.memset(colacc_raw[:], -60000.0)
        nc.gpsimd.memset(rs_t[:], 0.0)
        nc.gpsimd.memset(rmv_t[:], -60000.0)

        # ---- main loop ----
        AW = HC - SEG   # ScalarE (softmin) width of a mixed half
        for pt in range(NT):
            for h in range(2):
                hg = 2 * pt + h
                mixed = _is_mixed(hg)
                ptile = psum.tile([128, HC], f32, tag="ptile",
                                  name=f"pt{pt}_{h}")
                for q in range(4):
                    nc.tensor.matmul(
                        ptile[:, q * CH:(q + 1) * CH],
                        axs[0:K, pt * PT:(pt + 1) * PT],
                        ays[0:K, h * HC + q * CH:h * HC + (q + 1) * CH],
                        start=True, stop=True,
                    )
                w = AW if mixed else HC
                exph = stg.tile([128, HC], bf, tag="exph", name=f"exph{hg}")
                if mixed:
                    # DVE consumes the last bank directly, queued FIRST so
                    # the PSUM buf release never waits on the ScalarE chain
                    nc.vector.tensor_tensor(
                        colacc_raw[:, h * HC + AW:(h + 1) * HC],
                        colacc_raw[:, h * HC + AW:(h + 1) * HC],
                        ptile[:, AW:HC], op=MAX)
                    nc.vector.tensor_scalar(
                        junk[:], ptile[:, AW:HC], 1.0, None, MULT,
                        op1=MAX, accum_out=rmv_t[:, hg:hg + 1])
                nc.scalar.activation(
                    exph[:, 0:w], ptile[:, 0:w],
                    mybir.ActivationFunctionType.Exp, scale=S,
                    accum_out=rs_t[:, hg:hg + 1])
                nc.vector.tensor_tensor(
                    colacc_exp[:, h * HC:h * HC + w],
                    colacc_exp[:, h * HC:h * HC + w],
                    exph[:, 0:w], op=MAX)

        # ---- KL term (tiny) ----
        mu2d = work.tile([128, LATENT // 128], f32, tag="mu2d")
        lv2d = work.tile([128, LATENT // 128], f32, tag="lv2d")
        nc.sync.dma_start(mu2d[:], mu.ap().rearrange("(p f) -> p f", p=128))
        nc.sync.dma_start(lv2d[:], lv.ap().rearrange("(p f) -> p f", p=128))
        klsq = work.tile([128, LATENT // 128], f32, tag="klsq")
        klex = work.tile([128, LATENT // 128], f32, tag="klex")
        klt = work.tile([128, LATENT // 128], f32, tag="klt")
        klp = work.tile([128, 1], f32, tag="klp")
        nc.vector.tensor_tensor(klsq[:], mu2d[:], mu2d[:], op=MULT)
        nc.scalar.activation(klex[:], lv2d[:], mybir.ActivationFunctionType.Exp)
        nc.vector.tensor_tensor(klt[:], lv2d[:], klsq[:],
                                op=mybir.AluOpType.subtract)
        nc.vector.tensor_tensor(klt[:], klt[:], klex[:],
                                op=mybir.AluOpType.subtract)
        nc.vector.reduce_sum(klp[:], klt[:], axis=mybir.AxisListType.X)
        nc.sync.dma_start(o_kl.ap(), klp[:])

        # ---- outputs ----
        nc.sync.dma_start(o_cexp.ap(), colacc_exp[:])
        nc.gpsimd.dma_start(o_craw.ap(), colacc_raw[:])
        nc.sync.dma_start(o_rs.ap(), rs_t[:])
        nc.gpsimd.dma_start(o_rmv.ap(), rmv_t[:])

    nc.compile()
    return nc


def _get_nc():
    if "nc" not in _cache:
        _cache["nc"] = _build_program()
    return _cache["nc"]


def _register_ntff_hook():
    import sys, types
    if "antenv.axon_hooks" in sys.modules:
        return
    try:
        from trn_agent_boot.trn_boot import _ntff_profile_via_ctypes
        hook = _ntff_profile_via_ctypes("/opt/axon/libaxon_pjrt.so")
        mod = types.ModuleType("antenv.axon_hooks")
        mod.get_axon_ntff_profile_hook = lambda: hook
        mod.set_axon_ntff_profile_hook = lambda h: None
        sys.modules["antenv.axon_hooks"] = mod
        from concourse import bass_utils
        bass_utils.upload_artifacts = lambda tmpdir: tmpdir
    except Exception:
        pass


def _run(in_maps, trace=False):
    from concourse.bass_utils import run_bass_kernel_spmd
    if trace:
        _register_ntff_hook()
    nc = _get_nc()
    return run_bass_kernel_spmd(nc, in_maps, list(range(NCORES)), trace=trace)


def _combine(results, recon_x, x):
    """Host-side finish: logs, rescue of underflowed rows/cols, means, KL."""
    thresh = np.exp(LN_THRESH)
    row_total = 0.0
    col_total = 0.0
    kl_sum = 0.0
    for c in range(NCORES):
        r = results[c]
        xs = recon_x[c].astype(np.float64)   # [3, N] row points
        ys = x[c].astype(np.float64)         # [3, N] col points

        # ---- rows: softmin part per half + exact raw segment (mixed) ----
        rs = r["o_rs"].astype(np.float64)    # [128, 64]
        rmv = r["o_rmv"].astype(np.float64)  # [128, 64]
        with np.errstate(divide="ignore"):
            d_soft = -np.log(np.maximum(rs, 1e-300)) / S     # [128, 64]
        d_raw = -rmv                         # 60000.0 where not mixed
        unreliable = rs < thresh
        d_soft_ok = np.where(unreliable, np.inf, d_soft)
        d_half_ok = np.minimum(d_soft_ok, d_raw)
        # [128, 32, 2] -> per-row min over the two halves
        dv = d_half_ok.reshape(128, NT, 2).min(2)            # [128, NT]
        any_unrel = unreliable.reshape(128, NT, 2).any(2)
        need = any_unrel & (dv > TAU0)
        # row index = pt*128 + p
        rowvals = np.ascontiguousarray(dv.transpose(1, 0)).reshape(N)
        if need.any():
            p_idx, t_idx = np.nonzero(need)
            idx = t_idx * PT + p_idx
            xr = xs[:, idx]                  # [3, R]
            d = ((xr * xr).sum(0)[:, None] + (ys * ys).sum(0)[None, :]
                 - 2.0 * xr.T @ ys)          # [R, N]
            rowvals[idx] = d.min(1)

        # ---- cols ----
        cexp = r["o_cexp"].astype(np.float64).max(0)   # [N]
        craw = r["o_craw"].astype(np.float64).max(0)   # [N]
        d_raw = -craw
        with np.errstate(divide="ignore"):
            d_exp = np.where(cexp > 0.0,
                             -np.log(np.maximum(cexp, 1e-300)) / S, np.inf)
        colvals = np.minimum(d_exp, d_raw)
        badc = (cexp < thresh) & (colvals > TAU0)
        if badc.any():
            idx = np.nonzero(badc)[0]
            yc = ys[:, idx]
            d = ((xs * xs).sum(0)[:, None] + (yc * yc).sum(0)[None, :]
                 - 2.0 * xs.T @ yc)          # [N, C]
            colvals[idx] = d.min(0)

        row_total += rowvals.mean()
        col_total += colvals.mean()
        kl_sum += r["o_kl"].astype(np.float64).sum()

    recon = (row_total + col_total) / NCORES
    kld = -0.5 * (B * LATENT * 1.0 + kl_sum) / B
    total = recon + BETA * kld
    return (np.float32(total), np.float32(recon), np.float32(kld))


def _prep_in_maps(recon_x, x, mu, logvar):
    in_maps = []
    for c in range(NCORES):
        AX, AY = build_aug(recon_x[c].astype(np.float64),
                           x[c].astype(np.float64))
        in_maps.append({"ax": AX, "ay": AY, "mu": mu[c], "lv": logvar[c]})
    return in_maps


def kernel(recon_x, x, mu, logvar, _trace=False):
    recon_x = np.ascontiguousarray(recon_x, dtype=np.float32)
    x = np.ascontiguousarray(x, dtype=np.float32)
    mu = np.ascontiguousarray(mu, dtype=np.float32)
    logvar = np.ascontiguousarray(logvar, dtype=np.float32)
    in_maps = _prep_in_maps(recon_x, x, mu, logvar)
    res = _run(in_maps, trace=_trace)
    out = _combine(res.results, recon_x, x)
    if _trace:
        return out, res
    return out
